# revision 1
# baseline (speedup 1.0000x reference)
"""Trainium2 Bass kernel for nn_CombinedRotaryEmbedding.

Math: the 32 sequential Givens rotations and the learned rotation_matrix
compose into a single 64x64 matrix M (host-precomputed).  The RoPE stage
  out_top = y1*cos - y2*sin ; out_bot = y1*sin + y2*cos
is rewritten as out = u ⊙ COS + w ⊙ SIN with
  u = x @ Mbig   (rows = [Y1 | Y2] per head-pair)
  w = x @ Msw    (rows = [-Y2 | Y1])
so no cross-partition data movement is needed on-device.

Sharding: sequence-parallel over 8 cores (1024 positions each).  The host
pre-transposes x to [core][128 partitions = (head%2, d_in)][b, head//2, s]
so the PE can contract over d_in with full 128-partition utilisation, and
inverse-permutes the output.  Device per core: 16 MB in + 16 MB out, DMA
roofline ~92 us @ 358 GB/s.
"""

import numpy as np


def _import_bass():
    try:
        import concourse.bass  # noqa: F401
    except ImportError:
        import sys

        sys.path.insert(0, "/opt/trn_rl_repo")


_import_bass()

import concourse.bass as bass  # noqa: E402
import concourse.mybir as mybir  # noqa: E402
from concourse.tile import TileContext  # noqa: E402
from concourse.vector_clock import ScopedClock  # noqa: E402

B, S, NSTATE = 4, 8192, 1024
H, D, NUM_ROT = 16, 64, 32
NCORES = 8
S_SH = S // NCORES  # 1024 positions per core
FREE = B * (H // 2) * S_SH  # 32768 columns per core
CHUNK = 4096  # x-columns per DMA (2 MB)
GROUP = 1024  # x-columns per PSUM group

F32 = mybir.dt.float32
F32R = mybir.dt.float32r


class _TileContextSplitDrain(TileContext):
    """TileContext whose final drain carries at most one sem wait per
    instruction — the walrus in this container rejects instructions
    with 2+ sync waits ("Too many sync wait commands")."""

    def _drain_and_barrier(self, tick_clock, wait_clock):
        nc = self.nc
        drain_inst = nc.sync.drain()
        wait_clock.add_sem_waits(
            drain_inst.ins, ScopedClock({None: tick_clock.global_clock})
        )
        si = drain_inst.ins.sync_info
        waits = list(si.on_wait or [])
        if len(waits) > 1:
            si.on_wait = [waits[0]]
            for w in waits[1:]:
                n = nc.sync.nop(nofuse=True, hint="drain_wait_split")
                n.ins.sync_info = type(si)(on_update=[], on_wait=[w])
        nc.all_engine_barrier()
        assert self.sems is not None
        popped = nc._tile_sem_poison_stack.pop()
        assert popped is self._sem_poison
        nc.clear_and_free_semaphores(list(self.sems.allocated().values()))
        nc.all_engine_barrier()


def _split_excess_waits(nc, limit=1):
    """Walrus here rejects instructions with >limit sync waits.  Hoist
    excess waits onto same-engine InstNoOps inserted immediately before
    the offending instruction (same engine stream => program order)."""
    n_split = 0
    for fn in nc.m.functions:
        for blk in fn.blocks:
            insts = blk.instructions
            i = 0
            while i < len(insts):
                inst = insts[i]
                si = getattr(inst, "sync_info", None)
                waits = list(si.on_wait) if (si and si.on_wait) else []
                if len(waits) > limit:
                    keep = waits[-limit:]
                    excess = waits[:-limit]
                    si.on_wait = keep
                    for j, w in enumerate(excess):
                        nop = mybir.InstNoOp(
                            name=f"{inst.name}-wsplit{j}",
                            engine=inst.engine,
                            bass_nofuse=True,
                            sync_info=mybir.SyncInfo(on_wait=[w], on_update=[]),
                        )
                        insts.insert(i, nop)
                        i += 1
                        n_split += 1
                i += 1
    return n_split


def compose_rotation(thetas: np.ndarray, rotation_matrix: np.ndarray) -> np.ndarray:
    """Fold the sequential Givens rotations + rotation_matrix into one 64x64."""
    M = np.eye(D, dtype=np.float64)
    th = thetas.astype(np.float64)
    for k in range(NUM_ROT):
        i, j = k % D, (k + 1) % D
        c, s = np.cos(th[k]), np.sin(th[k])
        mi = M[:, i] * c + M[:, j] * s
        mj = -M[:, i] * s + M[:, j] * c
        M[:, i], M[:, j] = mi, mj
    return M @ rotation_matrix.astype(np.float64)


def build_weights(thetas: np.ndarray, rotation_matrix: np.ndarray):
    """Mbig (u = [Y1|Y2]) and Msw (w = [-Y2|Y1]) as [k=128, m=128] fp32."""
    M64 = compose_rotation(thetas, rotation_matrix)
    Mev = M64[:, 0::2]  # y1 columns [64, 32]
    Mod = M64[:, 1::2]  # y2 columns
    Mbig = np.zeros((128, 128), dtype=np.float64)
    Msw = np.zeros((128, 128), dtype=np.float64)
    for hp in (0, 1):
        r = slice(hp * 64, hp * 64 + 64)
        c1 = slice(hp * 32, hp * 32 + 32)
        c2 = slice(64 + hp * 32, 64 + hp * 32 + 32)
        Mbig[r, c1] = Mev
        Mbig[r, c2] = Mod
        Msw[r, c1] = -Mod
        Msw[r, c2] = Mev
    return Mbig.astype(np.float32), Msw.astype(np.float32)


def build_tables(inv_freq: np.ndarray):
    """Per-core [128, 2048] tables, free = [cos(s 0:512)|sin(0:512)|cos(512:1024)|sin(...)].

    Row p uses inv_freq[p % 32].  Args are computed in fp32 to match the
    reference's fp32 `pos * inv_freq` rounding (matters at s ~ 8191 where
    fp32 arg rounding shifts sin/cos by up to ~1e-3).
    """
    invf = inv_freq.astype(np.float32)
    tabs = np.empty((NCORES, 128, 2048), dtype=np.float32)
    l = np.arange(128) % 32

    # Reproduce the reference's sin/cos bit-for-bit: the jax stack in this
    # environment lowers sin/cos through neuronxcc with its own argument
    # reduction (up to ~4.5e-4 from libm at args ~8000 rad).  Mirror the
    # reference's exact ops; fall back to numpy if jax is unavailable.
    try:
        import jax.numpy as jnp

        pos = jnp.arange(S, dtype=jnp.float32)
        sinusoid = pos[:, None] * jnp.asarray(invf)[None, :]  # [S, 32]
        sin_all = np.asarray(jnp.sin(sinusoid))
        cos_all = np.asarray(jnp.cos(sinusoid))
    except Exception:
        args = np.arange(S, dtype=np.float32)[:, None] * invf[None, :]
        sin_all, cos_all = np.sin(args), np.cos(args)

    for c in range(NCORES):
        sl = slice(c * S_SH, (c + 1) * S_SH)
        cosv = cos_all[sl].T[l]  # [128, 1024]
        sinv = sin_all[sl].T[l]
        tabs[c, :, 0:512] = cosv[:, 0:512]
        tabs[c, :, 512:1024] = sinv[:, 0:512]
        tabs[c, :, 1024:1536] = cosv[:, 512:1024]
        tabs[c, :, 1536:2048] = sinv[:, 512:1024]
    return tabs


def shard_x(x: np.ndarray) -> np.ndarray:
    """[B,S,1024] -> [core, 128 (hp,d), FREE (b,hi,s)] contiguous."""
    xr = np.ascontiguousarray(x).reshape(B, NCORES, S_SH, H // 2, 2, D)
    xt = xr.transpose(1, 4, 5, 0, 3, 2)  # (core, hp, d, b, hi, sl)
    return np.ascontiguousarray(xt).reshape(NCORES, 128, FREE)


def unshard_out(o: np.ndarray) -> np.ndarray:
    """[core, 128 (half,hp,l), FREE (b,hi,s)] -> [B,S,1024]."""
    orr = o.reshape(NCORES, 2, 2, 32, B, H // 2, S_SH)
    ot = orr.transpose(4, 0, 6, 5, 2, 1, 3)  # (b, core, sl, hi, hp, half, l)
    return np.ascontiguousarray(ot).reshape(B, S, NSTATE)


_NC_CACHE = {}


def _build_nc():
    if "nc" in _NC_CACHE:
        return _NC_CACHE["nc"]
    nc = bass.Bass(trn_type="TRN2")
    x_d = nc.dram_tensor("x", [128, FREE], F32R, kind="ExternalInput")
    mb_d = nc.dram_tensor("mb", [128, 128], F32R, kind="ExternalInput")
    msw_d = nc.dram_tensor("msw", [128, 128], F32R, kind="ExternalInput")
    tab_d = nc.dram_tensor("tab", [128, 2048], F32, kind="ExternalInput")
    o_d = nc.dram_tensor("o", [128, FREE], F32, kind="ExternalOutput")

    with _TileContextSplitDrain(nc) as tc:
        with tc.tile_pool(name="const", bufs=1) as cpool, \
             tc.tile_pool(name="xin", bufs=3) as xpool, \
             tc.tile_pool(name="t12", bufs=3) as tpool, \
             tc.tile_pool(name="oout", bufs=2) as opool, \
             tc.tile_pool(name="psum", bufs=2, space="PSUM") as ppool:
            mb = cpool.tile([128, 128], F32R, tag="mb")
            msw = cpool.tile([128, 128], F32R, tag="msw")
            tab = cpool.tile([128, 2048], F32, tag="tab")
            nc.sync.dma_start(out=mb, in_=mb_d.ap())
            nc.sync.dma_start(out=msw, in_=msw_d.ap())
            nc.sync.dma_start(out=tab, in_=tab_d.ap())

            for ch in range(FREE // CHUNK):
                xt = xpool.tile([128, CHUNK], F32R)
                nc.sync.dma_start(
                    out=xt, in_=x_d.ap()[:, ch * CHUNK : (ch + 1) * CHUNK]
                )
                ot = opool.tile([128, CHUNK], F32)
                for g in range(CHUNK // GROUP):
                    base = g * GROUP
                    ps = ppool.tile([128, 2048], F32)
                    for sub in range(2):
                        xs = xt[:, base + sub * 512 : base + (sub + 1) * 512]
                        nc.tensor.matmul(
                            ps[:, sub * 1024 : sub * 1024 + 512],
                            lhsT=mb, rhs=xs, start=True, stop=True,
                        )
                        nc.tensor.matmul(
                            ps[:, sub * 1024 + 512 : sub * 1024 + 1024],
                            lhsT=msw, rhs=xs, start=True, stop=True,
                        )
                    t12 = tpool.tile([128, 2048], F32)
                    nc.vector.tensor_mul(out=t12, in0=ps, in1=tab)
                    v = t12.rearrange("p (g two f) -> p g two f", g=2, two=2)
                    ov = ot[:, base : base + GROUP].rearrange(
                        "p (g f) -> p g f", g=2
                    )
                    nc.gpsimd.tensor_add(
                        out=ov, in0=v[:, :, 0, :], in1=v[:, :, 1, :]
                    )
                nc.sync.dma_start(
                    out=o_d.ap()[:, ch * CHUNK : (ch + 1) * CHUNK], in_=ot
                )
    _split_excess_waits(nc)
    _NC_CACHE["nc"] = nc
    return nc


def kernel(x, thetas, rotation_matrix, inv_freq, _trace=False):
    from concourse.bass_utils import run_bass_kernel_spmd

    x = np.asarray(x, dtype=np.float32)
    thetas = np.asarray(thetas, dtype=np.float32)
    rotation_matrix = np.asarray(rotation_matrix, dtype=np.float32)
    inv_freq = np.asarray(inv_freq, dtype=np.float32)

    Mbig, Msw = build_weights(thetas, rotation_matrix)
    tabs = build_tables(inv_freq)
    xs = shard_x(x)

    nc = _build_nc()
    in_maps = [
        {"x": xs[c], "mb": Mbig, "msw": Msw, "tab": tabs[c]} for c in range(NCORES)
    ]
    res = run_bass_kernel_spmd(
        nc, in_maps, core_ids=list(range(NCORES)), trace=_trace
    )
    o = np.stack([res.results[c]["o"] for c in range(NCORES)])
    out = unshard_out(o)
    if _trace:
        return out, res
    return out



# revision 5
# speedup vs baseline: 1.0314x; 1.0314x over previous
"""Trainium2 Bass kernel for nn_CombinedRotaryEmbedding.

Math: the 32 sequential Givens rotations and the learned rotation_matrix
compose into a single 64x64 matrix M (host-precomputed).  The RoPE stage
  out_top = y1*cos - y2*sin ; out_bot = y1*sin + y2*cos
is rewritten as out = u . COS + w . SIN with
  u = x @ Mbig   (rows = [Y1 | Y2] per head-pair)
  w = x @ Msw    (rows = [-Y2 | Y1])
so no cross-partition data movement is needed on-device.

v2: fp16 device I/O (tolerance 2e-2 >> fp16's ~6e-4) halves HBM traffic
to ~17 MB/core (~50 us roofline @ 358 GB/s).  The elementwise stage is
split across three engines so each stays near the DMA floor (GPSIMD
cannot touch PSUM on TRN2, and DVE TensorTensor reaches 2x only when
every operand is a packed 2-byte dtype, so PSUM cols must be drained
to SBUF fp16 first; Act drains at 0.83 ns/col):
  - Act  : PSUM->SBUF fp16 copy of cols [0:1760)  (all of u + most of w)
  - DVE  : fp16 2x mult of cols [0:1412), 1x PSUM mult of [1760:2048),
           fp16 2x add of out cols [0:624)
  - Pool : fp16 mult of drained cols [1412:1760), add of out [624:1024)
Per-group model: Act 1609 ns, DVE 1604 ns, Pool 1604 ns.

Sharding: sequence-parallel over 8 cores (1024 positions each).  The host
pre-transposes x to [core][128 partitions = (head%2, d_in)][b, head//2, s]
so the PE can contract over d_in with full 128-partition utilisation, and
inverse-permutes the output.
"""

import numpy as np


def _import_bass():
    try:
        import concourse.bass  # noqa: F401
    except ImportError:
        import sys

        sys.path.insert(0, "/opt/trn_rl_repo")


_import_bass()

import concourse.bass as bass  # noqa: E402
import concourse.mybir as mybir  # noqa: E402
from concourse.tile import TileContext  # noqa: E402
from concourse.vector_clock import ScopedClock  # noqa: E402

B, S, NSTATE = 4, 8192, 1024
H, D, NUM_ROT = 16, 64, 32
NCORES = 8
S_SH = S // NCORES  # 1024 positions per core
FREE = B * (H // 2) * S_SH  # 32768 columns per core
CHUNK = 4096  # x-columns per DMA (1 MB fp16)
GROUP = 1024  # x-columns per PSUM group
DR = 1760  # cols [0:DR) of each [u|w] group drained via Act copy
DVE_M = 1412  # drained cols [0:DVE_M) multiplied on DVE at fp16 2x
ADD_SPLIT = 624  # out cols [0:ADD_SPLIT) added on DVE, rest on Pool

F32 = mybir.dt.float32
F16 = mybir.dt.float16


class _TileContextSplitDrain(TileContext):
    """TileContext whose final drain carries at most one sem wait per
    instruction — the walrus in this container rejects instructions
    with 2+ sync waits ("Too many sync wait commands")."""

    def _drain_and_barrier(self, tick_clock, wait_clock):
        nc = self.nc
        drain_inst = nc.sync.drain()
        wait_clock.add_sem_waits(
            drain_inst.ins, ScopedClock({None: tick_clock.global_clock})
        )
        si = drain_inst.ins.sync_info
        waits = list(si.on_wait or [])
        if len(waits) > 1:
            si.on_wait = [waits[0]]
            for w in waits[1:]:
                n = nc.sync.nop(nofuse=True, hint="drain_wait_split")
                n.ins.sync_info = type(si)(on_update=[], on_wait=[w])
        nc.all_engine_barrier()
        assert self.sems is not None
        popped = nc._tile_sem_poison_stack.pop()
        assert popped is self._sem_poison
        nc.clear_and_free_semaphores(list(self.sems.allocated().values()))
        nc.all_engine_barrier()


def _split_excess_waits(nc, limit=1):
    """Walrus here rejects instructions with >limit sync waits.  Hoist
    excess waits onto same-engine InstNoOps inserted immediately before
    the offending instruction (same engine stream => program order)."""
    n_split = 0
    for fn in nc.m.functions:
        for blk in fn.blocks:
            insts = blk.instructions
            i = 0
            while i < len(insts):
                inst = insts[i]
                si = getattr(inst, "sync_info", None)
                waits = list(si.on_wait) if (si and si.on_wait) else []
                if len(waits) > limit:
                    keep = waits[-limit:]
                    excess = waits[:-limit]
                    si.on_wait = keep
                    for j, w in enumerate(excess):
                        nop = mybir.InstNoOp(
                            name=f"{inst.name}-wsplit{j}",
                            engine=inst.engine,
                            bass_nofuse=True,
                            sync_info=mybir.SyncInfo(on_wait=[w], on_update=[]),
                        )
                        insts.insert(i, nop)
                        i += 1
                        n_split += 1
                i += 1
    return n_split


def compose_rotation(thetas: np.ndarray, rotation_matrix: np.ndarray) -> np.ndarray:
    """Fold the sequential Givens rotations + rotation_matrix into one 64x64."""
    M = np.eye(D, dtype=np.float64)
    th = thetas.astype(np.float64)
    for k in range(NUM_ROT):
        i, j = k % D, (k + 1) % D
        c, s = np.cos(th[k]), np.sin(th[k])
        mi = M[:, i] * c + M[:, j] * s
        mj = -M[:, i] * s + M[:, j] * c
        M[:, i], M[:, j] = mi, mj
    return M @ rotation_matrix.astype(np.float64)


def build_weights(thetas: np.ndarray, rotation_matrix: np.ndarray):
    """Mbig (u = [Y1|Y2]) and Msw (w = [-Y2|Y1]) as [k=128, m=128] fp16."""
    M64 = compose_rotation(thetas, rotation_matrix)
    Mev = M64[:, 0::2]  # y1 columns [64, 32]
    Mod = M64[:, 1::2]  # y2 columns
    Mbig = np.zeros((128, 128), dtype=np.float64)
    Msw = np.zeros((128, 128), dtype=np.float64)
    for hp in (0, 1):
        r = slice(hp * 64, hp * 64 + 64)
        c1 = slice(hp * 32, hp * 32 + 32)
        c2 = slice(64 + hp * 32, 64 + hp * 32 + 32)
        Mbig[r, c1] = Mev
        Mbig[r, c2] = Mod
        Msw[r, c1] = -Mod
        Msw[r, c2] = Mev
    return Mbig.astype(np.float16), Msw.astype(np.float16)


def build_tables(inv_freq: np.ndarray):
    """Per-core [128, 2048] fp16 tables, free = [cos(s 0:1024) | sin(0:1024)].

    Row p uses inv_freq[p % 32].  Args are computed in fp32 to match the
    reference's fp32 `pos * inv_freq` rounding.
    """
    invf = inv_freq.astype(np.float32)
    tabs = np.empty((NCORES, 128, 2048), dtype=np.float16)
    l = np.arange(128) % 32

    # Reproduce the reference's sin/cos: the jax stack in this environment
    # lowers sin/cos through neuronxcc with its own argument reduction (up
    # to ~4.5e-4 from libm at args ~8000 rad).  Mirror the reference's
    # exact ops; fall back to numpy if jax is unavailable.
    try:
        import jax.numpy as jnp

        pos = jnp.arange(S, dtype=jnp.float32)
        sinusoid = pos[:, None] * jnp.asarray(invf)[None, :]  # [S, 32]
        sin_all = np.asarray(jnp.sin(sinusoid))
        cos_all = np.asarray(jnp.cos(sinusoid))
    except Exception:
        args = np.arange(S, dtype=np.float32)[:, None] * invf[None, :]
        sin_all, cos_all = np.sin(args), np.cos(args)

    for c in range(NCORES):
        sl = slice(c * S_SH, (c + 1) * S_SH)
        tabs[c, :, 0:1024] = cos_all[sl].T[l]
        tabs[c, :, 1024:2048] = sin_all[sl].T[l]
    return tabs


def shard_x(x: np.ndarray) -> np.ndarray:
    """[B,S,1024] f32 -> [core, 128 (hp,d), FREE (b,hi,s)] fp16 contiguous."""
    xr = np.ascontiguousarray(x).reshape(B, NCORES, S_SH, H // 2, 2, D)
    xt = xr.transpose(1, 4, 5, 0, 3, 2)  # (core, hp, d, b, hi, sl)
    return np.ascontiguousarray(xt, dtype=np.float16).reshape(NCORES, 128, FREE)


def unshard_out(o: np.ndarray) -> np.ndarray:
    """[core, 128 (half,hp,l), FREE (b,hi,s)] fp16 -> [B,S,1024] f32."""
    orr = o.astype(np.float32).reshape(NCORES, 2, 2, 32, B, H // 2, S_SH)
    ot = orr.transpose(4, 0, 6, 5, 2, 1, 3)  # (b, core, sl, hi, hp, half, l)
    return np.ascontiguousarray(ot).reshape(B, S, NSTATE)


_NC_CACHE = {}


def _build_nc():
    if "nc" in _NC_CACHE:
        return _NC_CACHE["nc"]
    nc = bass.Bass(trn_type="TRN2")
    x_d = nc.dram_tensor("x", [128, FREE], F16, kind="ExternalInput")
    mb_d = nc.dram_tensor("mb", [128, 128], F16, kind="ExternalInput")
    msw_d = nc.dram_tensor("msw", [128, 128], F16, kind="ExternalInput")
    tab_d = nc.dram_tensor("tab", [128, 2048], F16, kind="ExternalInput")
    o_d = nc.dram_tensor("o", [128, FREE], F16, kind="ExternalOutput")

    with _TileContextSplitDrain(nc) as tc:
        with tc.tile_pool(name="const", bufs=1) as cpool, \
             tc.tile_pool(name="xin", bufs=3) as xpool, \
             tc.tile_pool(name="us", bufs=3) as upool, \
             tc.tile_pool(name="t12", bufs=3) as tpool, \
             tc.tile_pool(name="oout", bufs=2) as opool, \
             tc.tile_pool(name="psum", bufs=2, space="PSUM") as ppool:
            mb = cpool.tile([128, 128], F16, tag="mb")
            msw = cpool.tile([128, 128], F16, tag="msw")
            tab = cpool.tile([128, 2048], F16, tag="tab")
            nc.sync.dma_start(out=mb, in_=mb_d.ap())
            nc.sync.dma_start(out=msw, in_=msw_d.ap())
            nc.sync.dma_start(out=tab, in_=tab_d.ap())

            for ch in range(FREE // CHUNK):
                xt = xpool.tile([128, CHUNK], F16)
                nc.sync.dma_start(
                    out=xt, in_=x_d.ap()[:, ch * CHUNK : (ch + 1) * CHUNK]
                )
                ot = opool.tile([128, CHUNK], F16)
                for g in range(CHUNK // GROUP):
                    base = g * GROUP
                    xs0 = xt[:, base : base + 512]
                    xs1 = xt[:, base + 512 : base + 1024]
                    # ps = [u(1024) | w(1024)]; mb,mb,msw,msw order keeps
                    # LDWEIGHTS at 2 per group.
                    ps = ppool.tile([128, 2048], F32)
                    nc.tensor.matmul(ps[:, 0:512], lhsT=mb, rhs=xs0,
                                     start=True, stop=True)
                    nc.tensor.matmul(ps[:, 512:1024], lhsT=mb, rhs=xs1,
                                     start=True, stop=True)
                    nc.tensor.matmul(ps[:, 1024:1536], lhsT=msw, rhs=xs0,
                                     start=True, stop=True)
                    nc.tensor.matmul(ps[:, 1536:2048], lhsT=msw, rhs=xs1,
                                     start=True, stop=True)

                    # Drain+multiply split (GPSIMD cannot read PSUM):
                    # Act copies [0:DR) to SBUF fp16; DVE 2x-mults
                    # [0:DVE_M), Pool mults [DVE_M:DR), DVE 1x-mults the
                    # PSUM remainder [DR:2048).
                    us = upool.tile([128, DR], F16)
                    nc.scalar.copy(out=us, in_=ps[:, 0:DR])
                    t12 = tpool.tile([128, 2048], F16)
                    nc.vector.tensor_mul(
                        out=t12[:, 0:DVE_M], in0=us[:, 0:DVE_M],
                        in1=tab[:, 0:DVE_M],
                    )
                    nc.gpsimd.tensor_mul(
                        out=t12[:, DVE_M:DR], in0=us[:, DVE_M:DR],
                        in1=tab[:, DVE_M:DR],
                    )
                    nc.vector.tensor_mul(
                        out=t12[:, DR:2048], in0=ps[:, DR:2048],
                        in1=tab[:, DR:2048],
                    )
                    # out = u.COS + w.SIN  (all-fp16 SBUF; split DVE/Pool)
                    nc.vector.tensor_add(
                        out=ot[:, base : base + ADD_SPLIT],
                        in0=t12[:, 0:ADD_SPLIT],
                        in1=t12[:, 1024 : 1024 + ADD_SPLIT],
                    )
                    nc.gpsimd.tensor_add(
                        out=ot[:, base + ADD_SPLIT : base + GROUP],
                        in0=t12[:, ADD_SPLIT:1024],
                        in1=t12[:, 1024 + ADD_SPLIT : 2048],
                    )
                nc.sync.dma_start(
                    out=o_d.ap()[:, ch * CHUNK : (ch + 1) * CHUNK], in_=ot
                )
    _split_excess_waits(nc)
    _NC_CACHE["nc"] = nc
    return nc


def kernel(x, thetas, rotation_matrix, inv_freq, _trace=False):
    from concourse.bass_utils import run_bass_kernel_spmd

    x = np.asarray(x, dtype=np.float32)
    thetas = np.asarray(thetas, dtype=np.float32)
    rotation_matrix = np.asarray(rotation_matrix, dtype=np.float32)
    inv_freq = np.asarray(inv_freq, dtype=np.float32)

    Mbig, Msw = build_weights(thetas, rotation_matrix)
    tabs = build_tables(inv_freq)
    xs = shard_x(x)

    nc = _build_nc()
    in_maps = [
        {"x": xs[c], "mb": Mbig, "msw": Msw, "tab": tabs[c]} for c in range(NCORES)
    ]
    res = run_bass_kernel_spmd(
        nc, in_maps, core_ids=list(range(NCORES)), trace=_trace
    )
    o = np.stack([res.results[c]["o"] for c in range(NCORES)])
    out = unshard_out(o)
    if _trace:
        return out, res
    return out
